# revision 4
# baseline (speedup 1.0000x reference)
"""Trainium2 Bass kernel for nn_GAT_47906065220065.

SSGConv (K=1, alpha=0.5) -> GATv2(12 heads, 12 dim) -> GATv2(1 head, 64 dim)
over a fixed random graph: N=100000 nodes, E=1000000 edges (+ self loops).

Distribution: nodes are relabeled by a degree-balanced permutation, then
destination nodes are sharded contiguously across the 8 cores (12800 per
core).  Edges live with their destination core, grouped into 128-dst blocks
padded to TP 128-edge tiles.

Per block (python-unrolled, all tiles batched into single instructions
where possible, bf16 data / fp32 PSUM accumulation):
  - one multi-index INDIRECT1D gathers all TP*128 source rows
  - one DVE is_equal (double-broadcast) builds all TP one-hot S tiles
  - St (the transposed selector for the xr-broadcast matmul) comes from a
    gpsimd partition_broadcast of the dst row + one DVE is_equal
  - per tile, one PE matmul broadcasts xr to edges (z += St_t^T @ xr) and
    one accumulates the segment sums (blk += S_t^T @ [a*xl | a])
  - the lrelu/att-dot/exp chain runs once per block over [128, TP*F]
  - the dense projections (x@W) are fused per 128-node block, so the
    gather tables xl1/xl2 are produced inline and only two AllGathers
    (of the projected tables) are needed between phases.
"""

import os
import sys

sys.path.insert(0, '/opt/trn_rl_repo')

import numpy as np
import ml_dtypes

import bass_rust
import concourse.bacc as bacc
import concourse.bass as bass
import concourse.mybir as mybir
import concourse.tile as tile
from concourse.bass_utils import run_bass_kernel_spmd

BF16 = ml_dtypes.bfloat16

# ---------------------------------------------------------------- sizes
N = 100000
NPAD = 102400
NCORES = 8
PERCORE = NPAD // NCORES          # 12800
NBLK = PERCORE // 128             # 100
NBINS = NPAD // 128               # 800
QROWS = NPAD // 4                 # dma_gather int16 table quarter
ES1 = 128                         # padded gather row (bf16) phases 1/5
ES3 = 256                         # padded gather row (bf16) phase 3
D_IN = 64
F1 = 144
H1, C1 = 12, 12
F2 = 64
SHIFT = 4.0
NEG = 0.2
FP = mybir.dt.float32
BF = mybir.dt.bfloat16
I32 = mybir.dt.int32

AF = mybir.ActivationFunctionType
ALU = mybir.AluOpType


# ------------------------------------------------- walrus compatibility
def _drain_and_barrier_split(self, tick_clock, wait_clock):
    """End-of-TileContext drain for a walrus build that accepts at most
    one sync wait per instruction: spread the global-clock waits over
    NoOps instead of piling them on the final drain."""
    from concourse.vector_clock import ScopedClock

    carrier = self.nc.sync.nop(nofuse=True)
    wait_clock.add_sem_waits(
        carrier.ins, ScopedClock({None: tick_clock.global_clock})
    )
    si0 = carrier.ins.sync_info
    waits = list(si0.on_wait or []) if si0 is not None else []
    if len(waits) > 1:
        carrier.ins.sync_info = bass_rust.SyncInfo(
            on_wait=waits[:1], on_update=list(si0.on_update or [])
        )
        for w in waits[1:]:
            extra = self.nc.sync.nop(nofuse=True)
            extra.ins.sync_info = bass_rust.SyncInfo(on_wait=[w], on_update=[])
    self.nc.sync.drain()

    self.nc.all_engine_barrier()
    assert self.sems is not None
    popped = self.nc._tile_sem_poison_stack.pop()
    assert popped is self._sem_poison
    self.nc.clear_and_free_semaphores(list(self.sems.allocated().values()))
    self.nc.all_engine_barrier()


tile.TileContext._drain_and_barrier = _drain_and_barrier_split

_WSPLIT_N = [0]


def _split_sync_waits(nc):
    """Move extra sync waits (this walrus allows 1/instruction) onto NoOps
    inserted before the over-subscribed instruction on the same engine."""
    def make_nop(engine, wait):
        _WSPLIT_N[0] += 1
        return mybir.InstNoOp(
            name=f"WSPLIT-{_WSPLIT_N[0]}", opcode="NoOp", engine=engine,
            debug=None, ins=[], outs=[], descendants=None,
            sync_info=bass_rust.SyncInfo(on_wait=[wait], on_update=[]),
            bass_sim_breakpoint=False, bass_priority=0,
            bass_wait_until_ts=None, bass_scheduled_tick=None,
            bass_scheduled_proc=None, bass_scheduled_scope=None,
            bass_addl_debug=None, text_hint=None, bass_nofuse=True,
        )

    for f in nc.m.functions:
        for bb in f.blocks:
            if not any(
                inst.sync_info and inst.sync_info.on_wait
                and len(inst.sync_info.on_wait) > 1
                for inst in bb.instructions
            ):
                continue
            new_insts = []
            for inst in bb.instructions:
                si = inst.sync_info
                waits = list(si.on_wait) if si and si.on_wait else []
                if len(waits) > 1:
                    for w in waits[:-1]:
                        new_insts.append(make_nop(inst.engine, w))
                    inst.sync_info = bass_rust.SyncInfo(
                        on_wait=[waits[-1]], on_update=list(si.on_update or [])
                    )
                new_insts.append(inst)
            bb.instructions = new_insts


# ------------------------------------------------------------ host prep
def _host_prep(features, edge_index, params):
    x = np.ascontiguousarray(np.asarray(features), dtype=np.float32)
    ei = np.asarray(edge_index)
    src = ei[0].astype(np.int64)
    dst = ei[1].astype(np.int64)

    s = np.concatenate([src, np.arange(N, dtype=np.int64)])
    d = np.concatenate([dst, np.arange(N, dtype=np.int64)])
    deg = np.bincount(d, minlength=N).astype(np.float32)
    dinv = 1.0 / np.sqrt(deg)
    norm = (dinv[s] * dinv[d]).astype(np.float32)

    # degree-balanced node relabeling: heaviest nodes round-robin over bins
    order = np.argsort(-deg, kind='stable')
    order_full = np.concatenate([order, np.arange(N, NPAD, dtype=np.int64)])
    newid = np.empty(NPAD, dtype=np.int64)
    ranks = np.arange(NPAD, dtype=np.int64)
    newid[order_full] = (ranks % NBINS) * 128 + ranks // NBINS

    xp = np.zeros((NPAD, D_IN), dtype=np.float32)
    xp[newid[:N]] = x

    sp = newid[s]
    dp = newid[d]
    blk = dp >> 7                  # global 128-dst block id
    dloc = (dp & 127).astype(np.float32)

    # sort edges by (dst block, src quarter); dma_gather uses int16 indices,
    # so sources are gathered per table quarter of QROWS rows.
    quarter = (sp // QROWS).astype(np.int64)
    eorder = np.argsort(blk * 4 + quarter, kind='stable')
    sp, dloc_s, norm_s = sp[eorder], dloc[eorder], norm[eorder]
    blk_s, q_s = blk[eorder], quarter[eorder]
    cnt_bq = np.zeros((NBINS, 4), dtype=np.int64)
    np.add.at(cnt_bq, (blk_s, q_s), 1)
    TQ = tuple(int(v) for v in (cnt_bq.max(axis=0) + 127) // 128)
    OFF = np.concatenate([[0], np.cumsum(TQ)])
    TP = int(OFF[4])

    cap = TP * 128
    srcs_p = np.zeros((NBINS, cap), dtype=np.int16)
    dstf_p = np.full((NBINS, cap), 255.0, dtype=np.float32)   # 255 = padding
    cnorm_p = np.zeros((NBINS, cap), dtype=np.float32)
    ofs_bq = np.concatenate([[0], np.cumsum(cnt_bq.ravel())])
    within_bq = np.arange(len(sp)) - ofs_bq[blk_s * 4 + q_s]
    slot = OFF[q_s] * 128 + within_bq
    srcs_p[blk_s, slot] = (sp - q_s * QROWS).astype(np.int16)
    dstf_p[blk_s, slot] = dloc_s
    cnorm_p[blk_s, slot] = norm_s

    # stream layout [128, NBLK*TP] per core: col b*TP+t, row p -> edge (b, t*128+p)
    def streams(arr, dt):
        a = arr.reshape(NCORES, NBLK, TP, 128)
        return [np.ascontiguousarray(
            a[c].transpose(2, 0, 1).reshape(128, NBLK * TP)).astype(dt)
            for c in range(NCORES)]

    dstf_c = streams(dstf_p, BF16)
    cnorm_c = streams(cnorm_p, BF16)
    # dst row layout [NBLK, TP*128]: dstrow[b, t*128+p] = dst of edge (b,t,p)
    dstrow_c = [np.ascontiguousarray(
        dstf_p.reshape(NCORES, NBLK, cap)[c]).astype(BF16)
        for c in range(NCORES)]
    # dma_gather int16 index stream: per (block, tile) 8 columns of 16-lane
    # wrapped indices, replicated across the 8 gpsimd cores (partition dim).
    # unwrapped[i] = idxs[i % 16, i // 16] with i the slot within the quarter.
    i16 = srcs_p.reshape(NCORES, NBLK, TP * 8, 16)
    idx16_c = [np.ascontiguousarray(np.tile(
        i16[c].transpose(0, 2, 1).reshape(NBLK, 16, TP * 8)
        .transpose(1, 0, 2).reshape(16, NBLK * TP * 8), (8, 1)))
        for c in range(NCORES)]

    g = lambda k: np.ascontiguousarray(np.asarray(params[k]), dtype=np.float32)
    W_ssg, b_ssg = g('W_ssg'), g('b_ssg')
    W1l, b1l, W1r, b1r = g('W1l'), g('b1l'), g('W1r'), g('b1r')
    att1, bias1 = g('att1'), g('bias1')
    W2l, b2l, W2r, b2r = g('W2l'), g('b2l'), g('W2r'), g('b2r')
    att2, bias2 = g('att2'), g('bias2')

    col = lambda v: np.ascontiguousarray(v.reshape(-1, 1), dtype=np.float32)
    b1sum = bias1 + b1l                 # softmax weights sum to 1 -> b1l
    b2sum = bias2 + b2l                 # moves to the output bias
    bf = lambda v: np.ascontiguousarray(v).astype(BF16)
    consts_bf = dict(
        iota=bf(np.tile(np.arange(128, dtype=np.float32), (128, 1))),
        ones1=bf(np.ones((1, 128), dtype=np.float32)),
        wssg05=bf(0.5 * W_ssg),
        w1l=bf(W1l),
        w1r=bf(W1r),
        b1r_row=bf((b1l + b1r).reshape(1, F1)),
        att1r=bf(np.tile(att1.reshape(1, F1), (128, 1))),
        w2lt=bf(W2l[:128, :]), w2lb=bf(W2l[128:, :]),
        w2rt=bf(W2r[:128, :]), w2rb=bf(W2r[128:, :]),
        b2r_row=bf((b2l + b2r).reshape(1, F2)),
        att2r=bf(np.tile(att2.reshape(1, F2), (128, 1))),
    )
    consts_fp = dict(
        iotac=col(np.arange(128, dtype=np.float32)),
        bssg=col(b_ssg),
        b1sum_a=col(b1sum[:128]), b1sum_b=col(b1sum[128:]),
        bias2rep=np.tile(b2sum.reshape(1, F2), (128, 1)).astype(np.float32),
        nshift=np.full((128, 1), -SHIFT, dtype=np.float32),
        ident=np.eye(128, dtype=np.float32),
    )

    xg_tab = np.zeros((NPAD, ES1), dtype=BF16)
    xg_tab[:, :D_IN] = xp.astype(BF16)
    xoT = np.ascontiguousarray(xp.T)    # [64, NPAD] fp32

    in_maps = []
    for c in range(NCORES):
        m = dict(consts_bf)
        m.update(consts_fp)
        m['xg'] = xg_tab
        m['xoT'] = np.ascontiguousarray(xoT[:, c * PERCORE:(c + 1) * PERCORE])
        m['idx16'] = idx16_c[c]
        m['dstf'] = dstf_c[c]
        m['cnorm'] = cnorm_c[c]
        m['dstrow'] = dstrow_c[c]
        in_maps.append(m)
    return in_maps, newid, TQ


# --------------------------------------------------------- kernel build
def _build(TQ):
    nc = bacc.Bacc()
    TP = int(sum(TQ))
    OFF = [0]
    for v in TQ:
        OFF.append(OFF[-1] + v)
    NT = NBLK * TP
    ds = bass.ds
    rg = [list(range(NCORES))]
    I16 = mybir.dt.int16

    xg = nc.declare_dram_parameter("xg", [NPAD, ES1], BF, isOutput=False)
    xoT = nc.declare_dram_parameter("xoT", [D_IN, PERCORE], FP, isOutput=False)
    idx16 = nc.declare_dram_parameter("idx16", [128, NT * 8], I16,
                                      isOutput=False)
    dstf = nc.declare_dram_parameter("dstf", [128, NT], BF, isOutput=False)
    cnorm = nc.declare_dram_parameter("cnorm", [128, NT], BF, isOutput=False)
    dstrow = nc.declare_dram_parameter("dstrow", [NBLK, TP * 128], BF,
                                       isOutput=False)

    cshape_bf = dict(
        iota=[128, 128], ones1=[1, 128], wssg05=[64, 64],
        w1l=[64, F1], w1r=[64, F1], b1r_row=[1, F1], att1r=[128, F1],
        w2lt=[128, F2], w2lb=[16, F2], w2rt=[128, F2], w2rb=[16, F2],
        b2r_row=[1, F2], att2r=[128, F2],
    )
    cshape_fp = dict(
        iotac=[128, 1], bssg=[64, 1], b1sum_a=[128, 1], b1sum_b=[16, 1],
        bias2rep=[128, F2], nshift=[128, 1], ident=[128, 128],
    )
    cparams = {}
    for k, v in cshape_bf.items():
        cparams[k] = nc.declare_dram_parameter(k, v, BF, isOutput=False)
    for k, v in cshape_fp.items():
        cparams[k] = nc.declare_dram_parameter(k, v, FP, isOutput=False)

    out = nc.declare_dram_parameter("out", [PERCORE, F2], FP, isOutput=True)

    x1T_loc = nc.dram_tensor("x1T_loc", [D_IN, PERCORE], BF)
    y1T_loc = nc.dram_tensor("y1T_loc", [F1, PERCORE], BF)
    xl1_loc = nc.dram_tensor("xl1_loc", [PERCORE, ES3], BF)
    xl1_all = nc.dram_tensor("xl1_all", [NPAD, ES3], BF, addr_space="Shared")
    xl2_loc = nc.dram_tensor("xl2_loc", [PERCORE, ES1], BF)
    xl2_all = nc.dram_tensor("xl2_all", [NPAD, ES1], BF, addr_space="Shared")

    # z-matmul PSUM groups per block: chunks of <=3 tiles (one bank each)
    ZCH = [(i, min(3, TP - i)) for i in range(0, TP, 3)]

    with tile.TileContext(nc) as tc:
        cpool = tc.alloc_tile_pool(name="consts", bufs=1)
        ct = {}
        for k, shp in cshape_bf.items():
            ct[k] = cpool.tile(shp, BF, tag=f"c_{k}", name=f"c_{k}")
            nc.sync.dma_start(out=ct[k][:], in_=cparams[k][:])
        for k, shp in cshape_fp.items():
            ct[k] = cpool.tile(shp, FP, tag=f"c_{k}", name=f"c_{k}")
            nc.sync.dma_start(out=ct[k][:], in_=cparams[k][:])
        # resident edge streams
        idx16_t = cpool.tile([128, NT * 8], I16, tag="idx16_t", name="idx16_t")
        nc.sync.dma_start(out=idx16_t[:], in_=idx16[:])
        dstf_t = cpool.tile([128, NT], BF, tag="dstf_t", name="dstf_t")
        nc.sync.dma_start(out=dstf_t[:], in_=dstf[:])
        cnorm_t = cpool.tile([128, NT], BF, tag="cnorm_t", name="cnorm_t")
        nc.sync.dma_start(out=cnorm_t[:], in_=cnorm[:])

        def bcast_col(ap_2d, inner):
            # [128, TP] slice -> [128, TP, inner] with stride-0 inner axis
            return ap_2d.rearrange("p (t o) -> p t o", o=1).to_broadcast(
                [128, TP, inner])

        def build_S(pool, b, tag):
            S = pool.tile([128, TP, 128], BF, tag=tag)
            nc.vector.tensor_tensor(
                out=S[:],
                in0=ct['iota'][:].rearrange("p (o d) -> p o d", o=1)
                    .to_broadcast([128, TP, 128]),
                in1=bcast_col(dstf_t[:, ds(b * TP, TP)], 128),
                op=ALU.is_equal)
            return S

        def build_St(pool, b):
            drow = pool.tile([1, TP * 128], BF, tag="drow")
            nc.sync.dma_start(out=drow[:], in_=dstrow[b:b + 1, :])
            drep = pool.tile([128, TP * 128], BF, tag="drep")
            nc.gpsimd.partition_broadcast(drep[:], drow[:])
            St = pool.tile([128, TP * 128], BF, tag="St")
            nc.vector.tensor_scalar(St[:], drep[:], ct['iotac'][:, :1], None,
                                    op0=ALU.is_equal)
            return St

        def gather(pool, b, table, ES, tag):
            gx = pool.tile([128, TP, ES], BF, tag=tag)
            for q in range(4):
                if TQ[q] == 0:
                    continue
                nc.gpsimd.dma_gather(
                    gx[:, OFF[q]:OFF[q] + TQ[q], :],
                    table[q * QROWS:(q + 1) * QROWS, :],
                    idx16_t[:, ds((b * TP + OFF[q]) * 8, TQ[q] * 8)],
                    TQ[q] * 128, TQ[q] * 128, ES)
            return gx

        # ---------------- phase 1: SSG conv ---------------------------
        with (tc.tile_pool(name="p1s", bufs=3) as pool,
              tc.tile_pool(name="p1a", bufs=2, space="PSUM") as ppa,
              tc.tile_pool(name="p1b", bufs=2, space="PSUM") as ppb,
              tc.tile_pool(name="p1c", bufs=2, space="PSUM") as ppc):
            for b in range(NBLK):
                gx = gather(pool, b, xg, ES1, "gx1")
                S = build_S(pool, b, "S1")
                gxn = pool.tile([128, TP, D_IN], BF, tag="gxn")
                nc.vector.tensor_tensor(
                    out=gxn[:], in0=gx[:, :, 0:D_IN],
                    in1=bcast_col(cnorm_t[:, ds(b * TP, TP)], D_IN),
                    op=ALU.mult)
                aggT = ppa.tile([64, 128], FP, tag="aggT")
                for t in range(TP):
                    nc.tensor.matmul(aggT[:], lhsT=gxn[:, t, :], rhs=S[:, t, :],
                                     start=(t == 0), stop=(t == TP - 1))
                xoT_s = pool.tile([64, 128], FP, tag="xoT_s")
                nc.sync.dma_start(out=xoT_s[:], in_=xoT[:, ds(b * 128, 128)])
                hTs = pool.tile([64, 128], BF, tag="hTs")
                nc.vector.tensor_tensor(out=hTs[:], in0=aggT[:], in1=xoT_s[:],
                                        op=ALU.add)
                x1P = ppb.tile([64, 128], FP, tag="x1P")
                nc.tensor.matmul(x1P[:], lhsT=ct['wssg05'][:], rhs=hTs[:],
                                 start=True, stop=True)
                x1s = pool.tile([64, 128], BF, tag="x1s")
                nc.scalar.activation(x1s[:], x1P[:], AF.Identity,
                                     bias=ct['bssg'][:, :1])
                nc.sync.dma_start(out=x1T_loc[:, ds(b * 128, 128)], in_=x1s[:])
                xl1P = ppc.tile([128, F1], FP, tag="xl1P")
                nc.tensor.matmul(xl1P[:], lhsT=x1s[:], rhs=ct['w1l'][:],
                                 start=True, stop=True)
                xl1s = pool.tile([128, F1], BF, tag="xl1s")
                nc.scalar.activation(xl1s[:], xl1P[:], AF.Copy)
                nc.sync.dma_start(out=xl1_loc[ds(b * 128, 128), 0:F1],
                                  in_=xl1s[:])

        nc.gpsimd.collective_compute(
            "AllGather", ALU.bypass, replica_groups=rg,
            ins=[xl1_loc[:]], outs=[xl1_all[:]])

        # ---------------- phase 3: GATv2 layer 1 ----------------------
        with (tc.tile_pool(name="p3s", bufs=2) as pool,
              tc.tile_pool(name="p3z", bufs=4, space="PSUM") as ppz,
              tc.tile_pool(name="p3b", bufs=1, space="PSUM") as ppb,
              tc.tile_pool(name="p3m", bufs=2, space="PSUM") as ppm,
              tc.tile_pool(name="p3t", bufs=1, space="PSUM") as ppt):
            for b in range(NBLK):
                gx = gather(pool, b, xl1_all, ES3, "gx3")
                S = build_S(pool, b, "S3")
                St = build_St(pool, b)
                x1b = pool.tile([64, 128], BF, tag="x1b")
                nc.sync.dma_start(out=x1b[:], in_=x1T_loc[:, ds(b * 128, 128)])
                xrdP = ppm.tile([128, F1], FP, tag="mm144")
                nc.tensor.matmul(xrdP[:], lhsT=x1b[:], rhs=ct['w1r'][:],
                                 start=True, stop=False)
                nc.tensor.matmul(xrdP[:], lhsT=ct['ones1'][:],
                                 rhs=ct['b1r_row'][:], start=False, stop=True)
                xrd = pool.tile([128, F1], BF, tag="xrd")
                nc.scalar.activation(xrd[:], xrdP[:], AF.Copy)

                zs = pool.tile([128, TP, F1], BF, tag="zs")
                for t0, gsz in ZCH:
                    zP = ppz.tile([128, 3, F1], FP, tag="zP")
                    for j in range(gsz):
                        t = t0 + j
                        nc.tensor.matmul(zP[:, j, :],
                                         lhsT=St[:, ds(t * 128, 128)],
                                         rhs=xrd[:], start=True, stop=True)
                    nc.vector.tensor_tensor(
                        out=zs[:, t0:t0 + gsz, :], in0=zP[:, 0:gsz, :],
                        in1=gx[:, t0:t0 + gsz, 0:F1], op=ALU.add)
                ab = pool.tile([128, TP, F1], BF, tag="ab")
                nc.scalar.activation(ab[:], zs[:], AF.Abs, scale=0.4)
                lr = pool.tile([128, TP, F1], BF, tag="lr")
                nc.vector.scalar_tensor_tensor(
                    out=lr[:], in0=zs[:], scalar=0.6, in1=ab[:],
                    op0=ALU.mult, op1=ALU.add)
                wm = pool.tile([128, TP, F1], BF, tag="wm")
                nc.vector.tensor_tensor(
                    out=wm[:], in0=lr[:],
                    in1=ct['att1r'][:].rearrange("p (o f) -> p o f", o=1)
                        .to_broadcast([128, TP, F1]),
                    op=ALU.mult)
                lg = pool.tile([128, TP, H1], FP, tag="lg")
                nc.vector.tensor_reduce(
                    out=lg[:], in_=wm[:].rearrange("p t (h c) -> p t h c", c=C1),
                    axis=mybir.AxisListType.X, op=ALU.add)
                ex = pool.tile([128, TP, H1], FP, tag="ex")
                nc.scalar.activation(ex[:], lg[:], AF.Exp,
                                     bias=ct['nshift'][:, :1])
                rhs = pool.tile([128, TP, F1 + H1], BF, tag="rhs")
                nc.vector.tensor_tensor(
                    out=rhs[:, :, 0:F1].rearrange("p t (h c) -> p t h c", c=C1),
                    in0=gx[:, :, 0:F1].rearrange("p t (h c) -> p t h c", c=C1),
                    in1=ex[:].rearrange("p t (h o) -> p t h o", o=1)
                        .to_broadcast([128, TP, H1, C1]),
                    op=ALU.mult)
                nc.vector.tensor_copy(out=rhs[:, :, F1:F1 + H1], in_=ex[:])
                blk = ppb.tile([128, F1 + H1], FP, tag="blk")
                for t in range(TP):
                    nc.tensor.matmul(blk[:], lhsT=S[:, t, :], rhs=rhs[:, t, :],
                                     start=(t == 0), stop=(t == TP - 1))
                den = pool.tile([128, H1], FP, tag="den")
                nc.vector.tensor_scalar(den[:], blk[:, F1:F1 + H1], 1e-16,
                                        None, op0=ALU.add)
                rec = pool.tile([128, H1], FP, tag="rec")
                nc.vector.reciprocal(rec[:], den[:])
                o1 = pool.tile([128, F1], FP, tag="o1")
                nc.vector.tensor_tensor(
                    out=o1[:].rearrange("p (h c) -> p h c", c=C1),
                    in0=blk[:, 0:F1].rearrange("p (h c) -> p h c", c=C1),
                    in1=rec[:].rearrange("p (h o) -> p h o", o=1)
                        .to_broadcast([128, H1, C1]),
                    op=ALU.mult)
                tY = ppt.tile([128, 256], FP, tag="tY")
                nc.tensor.transpose(out=tY[:, 0:128], in_=o1[:, 0:128],
                                    identity=ct['ident'][:])
                nc.tensor.transpose(out=tY[0:16, 128:256], in_=o1[:, 128:F1],
                                    identity=ct['ident'][:])
                sY1 = pool.tile([128, 128], BF, tag="sY1")
                nc.scalar.activation(sY1[:], tY[:, 0:128], AF.Identity,
                                     bias=ct['b1sum_a'][:, :1])
                sY2 = pool.tile([16, 128], BF, tag="sY2")
                nc.scalar.activation(sY2[:], tY[0:16, 128:256], AF.Identity,
                                     bias=ct['b1sum_b'][:, :1])
                nc.sync.dma_start(out=y1T_loc[0:128, ds(b * 128, 128)],
                                  in_=sY1[:])
                nc.sync.dma_start(out=y1T_loc[128:F1, ds(b * 128, 128)],
                                  in_=sY2[:])
                xl2P = ppm.tile([128, F1], FP, tag="mm144")
                nc.tensor.matmul(xl2P[:, 0:F2], lhsT=sY1[:], rhs=ct['w2lt'][:],
                                 start=True, stop=False)
                nc.tensor.matmul(xl2P[:, 0:F2], lhsT=sY2[:], rhs=ct['w2lb'][:],
                                 start=False, stop=True)
                xl2s = pool.tile([128, F2], BF, tag="xl2s")
                nc.scalar.activation(xl2s[:], xl2P[:, 0:F2], AF.Copy)
                nc.sync.dma_start(out=xl2_loc[ds(b * 128, 128), 0:F2],
                                  in_=xl2s[:])

        nc.gpsimd.collective_compute(
            "AllGather", ALU.bypass, replica_groups=rg,
            ins=[xl2_loc[:]], outs=[xl2_all[:]])

        # ---------------- phase 5: GATv2 layer 2 -> out ---------------
        with (tc.tile_pool(name="p5s", bufs=2) as pool,
              tc.tile_pool(name="p5z", bufs=4, space="PSUM") as ppz,
              tc.tile_pool(name="p5b", bufs=2, space="PSUM") as ppb,
              tc.tile_pool(name="p5m", bufs=2, space="PSUM") as ppm):
            for b in range(NBLK):
                gx = gather(pool, b, xl2_all, ES1, "gx5")
                S = build_S(pool, b, "S5")
                St = build_St(pool, b)
                y1b1 = pool.tile([128, 128], BF, tag="y1b1")
                nc.sync.dma_start(out=y1b1[:],
                                  in_=y1T_loc[0:128, ds(b * 128, 128)])
                y1b2 = pool.tile([16, 128], BF, tag="y1b2")
                nc.sync.dma_start(out=y1b2[:],
                                  in_=y1T_loc[128:F1, ds(b * 128, 128)])
                xrdP = ppm.tile([128, F2], FP, tag="mm64")
                nc.tensor.matmul(xrdP[:], lhsT=y1b1[:], rhs=ct['w2rt'][:],
                                 start=True, stop=False)
                nc.tensor.matmul(xrdP[:], lhsT=y1b2[:], rhs=ct['w2rb'][:],
                                 start=False, stop=False)
                nc.tensor.matmul(xrdP[:], lhsT=ct['ones1'][:],
                                 rhs=ct['b2r_row'][:], start=False, stop=True)
                xrd = pool.tile([128, F2], BF, tag="xrd5")
                nc.scalar.activation(xrd[:], xrdP[:], AF.Copy)

                zs = pool.tile([128, TP, F2], BF, tag="zs5")
                for t0, gsz in ZCH:
                    zP = ppz.tile([128, 3, F2], FP, tag="zP5")
                    for j in range(gsz):
                        t = t0 + j
                        nc.tensor.matmul(zP[:, j, :],
                                         lhsT=St[:, ds(t * 128, 128)],
                                         rhs=xrd[:], start=True, stop=True)
                    nc.vector.tensor_tensor(
                        out=zs[:, t0:t0 + gsz, :], in0=zP[:, 0:gsz, :],
                        in1=gx[:, t0:t0 + gsz, 0:F2], op=ALU.add)
                ab = pool.tile([128, TP, F2], BF, tag="ab5")
                nc.scalar.activation(ab[:], zs[:], AF.Abs, scale=0.4)
                lr = pool.tile([128, TP, F2], BF, tag="lr5")
                nc.vector.scalar_tensor_tensor(
                    out=lr[:], in0=zs[:], scalar=0.6, in1=ab[:],
                    op0=ALU.mult, op1=ALU.add)
                wm = pool.tile([128, TP, F2], BF, tag="wm5")
                nc.vector.tensor_tensor(
                    out=wm[:], in0=lr[:],
                    in1=ct['att2r'][:].rearrange("p (o f) -> p o f", o=1)
                        .to_broadcast([128, TP, F2]),
                    op=ALU.mult)
                lg = pool.tile([128, TP, 1], FP, tag="lg5")
                nc.vector.tensor_reduce(
                    out=lg[:], in_=wm[:], axis=mybir.AxisListType.X, op=ALU.add)
                ex = pool.tile([128, TP, 1], FP, tag="ex5")
                nc.scalar.activation(ex[:], lg[:], AF.Exp,
                                     bias=ct['nshift'][:, :1])
                rhs = pool.tile([128, TP, F2 + 1], BF, tag="rhs5")
                nc.vector.tensor_tensor(
                    out=rhs[:, :, 0:F2], in0=gx[:, :, 0:F2],
                    in1=ex[:].to_broadcast([128, TP, F2]), op=ALU.mult)
                nc.vector.tensor_copy(out=rhs[:, :, F2:F2 + 1], in_=ex[:])
                blk = ppb.tile([128, F2 + 1], FP, tag="blk5")
                for t in range(TP):
                    nc.tensor.matmul(blk[:], lhsT=S[:, t, :], rhs=rhs[:, t, :],
                                     start=(t == 0), stop=(t == TP - 1))
                den = pool.tile([128, 1], FP, tag="den5")
                nc.vector.tensor_scalar(den[:], blk[:, F2:F2 + 1], 1e-16,
                                        None, op0=ALU.add)
                rec = pool.tile([128, 1], FP, tag="rec5")
                nc.vector.reciprocal(rec[:], den[:])
                o2 = pool.tile([128, F2], FP, tag="o2")
                nc.vector.scalar_tensor_tensor(
                    out=o2[:], in0=blk[:, 0:F2], scalar=rec[:, :1],
                    in1=ct['bias2rep'][:], op0=ALU.mult, op1=ALU.add)
                nc.sync.dma_start(out=out[ds(b * 128, 128), :], in_=o2[:])

        cpool.release()

    nc.compile()
    _split_sync_waits(nc)
    return nc


_NC_CACHE = {}


def kernel(**inputs):
    features = inputs["features"]
    edge_index = inputs["edge_index"]
    in_maps, newid, TQ = _host_prep(features, edge_index, inputs)
    if TQ not in _NC_CACHE:
        _NC_CACHE[TQ] = _build(TQ)
    nc = _NC_CACHE[TQ]
    res = run_bass_kernel_spmd(nc, in_maps, list(range(NCORES)))
    y_new = np.concatenate([res.results[c]["out"] for c in range(NCORES)], axis=0)
    return np.ascontiguousarray(y_new[newid[:N]]).astype(np.float32)
